# revision 9
# baseline (speedup 1.0000x reference)
"""Trainium2 Bass kernel for nn_BRCLoss (supervised-contrastive style loss).

Math (per batch sample b, matching the jax reference):
    f = features[b].reshape(24, 4096); fhat = f / ||f||_row
    logits = (fhat @ fhat.T) / 0.1                       # [24, 24]
    exp_logits = exp(logits) * (1 - I)
    log_prob = logits - log(exp_logits.sum(-1))
    mlpp = (mask * log_prob).sum(-1) / (mask.sum(-1) + 1e-6)
    loss = sum_b mean_m(-0.1 * mlpp) / 512               # scalar

`outputs` / `targets` are unused by the reference; only `features`
[512, 2, 12, 4096] f32 matters.  Pure data parallel: 64 samples per core,
per-core partial results combined on the host.

Per-core kernel (HBM-read roofline ~67 us at ~375 GB/s busy-rate):
  - 12 tiles of [120 rows, 4096] (5 samples) + 1 tail tile of [96 rows]
    (4 samples) — 1536 rows exactly, nothing re-read.
  - ALL feature-load triggers are issued up front (fpool holds all 13
    tiles, ~104 KB/partition) so the 16 SWDGE engines never starve on
    descriptor supply: previously loads were sequenced inside the compute
    loop and the stream stretched ~9 us when compute lagged.
  - Feature loads are SWDGE (gpsimd) DMAs that cast f32 -> bf16 in flight:
    HBM still reads the full f32 stream (the memory roofline), but SBUF
    writes halve, which relieves the port bottleneck shared with the
    sibling NeuronCore under 8-core SPMD.
  - Per tile: PE-transposes 32 bf16 chunks [R,128] -> PSUM (8 per bank),
    copies them to SBUF (copies spread over DVE/ACT/Pool), then 32
    accumulating bf16 matmuls build the block-diagonal Gram G [R,R].
  - Epilogue: d2 = 0.1*diag(G) via a pre-scaled identity mask;
    rnx = d2^-0.5 as exp(-0.5*ln(d2)); L = diag(rnx) @ G @ diag(rnx) with
    the column scale on the PE (G symmetric) and the row scale as a
    per-partition tensor_scalar; the denominator row-sum is fused into the
    Exp activation via accum_out after adding a {0,-1e30} log-mask; the
    positive-pair sum is taken from the PSUM product directly (pm * h).
  - Per-tile results land as columns of one [120, 26] tile (sum(mask *
    logits) and ln(denominator) per tile); a single DMA ships it out and
    the HOST applies the -T/((msum+eps)*M*B) weights and reduces, so the
    device-side tail after the last tile is just one small DMA.
  - All activations (Ln/Exp/Copy) are pinned to the single
    natural_log_exp_and_others ACT table set (see _OneActSetBacc) so the
    kernel pays exactly one ACT_TABLE_LOAD instead of ~1.3 us per Ln<->Exp
    alternation.
"""

import os
import sys

import numpy as np

if "/opt/trn_rl_repo" not in sys.path:
    sys.path.insert(0, "/opt/trn_rl_repo")

# Problem constants (hardcoded; kernel.py must be self-contained).
B = 512
NV = 2
NCLS = 12
D = 4096
M = NV * NCLS              # 24 anchor rows per sample
NCORES = 8
SPC = B // NCORES          # 64 samples per core
ROWS = SPC * M             # 1536 feature rows per core
P = 120                    # rows per full tile (5 samples)
T = 13                     # tiles per core: 12 full + 1 tail of 96 rows
PTAIL = ROWS - P * (T - 1)  # 96 rows (4 samples) in the tail tile
CH = 128                   # contraction chunk (PE partition limit)
NCH = D // CH              # 32 chunks
QUAD = 8                   # transposed chunks packed per PSUM bank
NQ = NCH // QUAD
TEMP = 0.1
EPS_POS = 1e-6
NEG = -1.0e30              # additive log-mask for excluded logits

_compiled = None           # (nc, const_in_map)
LAST_RESULTS = None        # BassKernelResults of the most recent run


def _host_consts():
    """Masks shared by every core (the per-core sample grid is identical)."""
    import ml_dtypes

    i = np.arange(NCLS)
    graph = (np.abs(i[:, None] - i[None, :]) <= 1).astype(np.float32)   # [12,12]
    eye24 = np.eye(M, dtype=np.float32)
    mask24 = np.tile(graph, (NV, NV)) * (1.0 - eye24)                    # positives
    g5 = P // M
    blk = np.kron(np.eye(g5, dtype=np.float32), np.ones((M, M), np.float32))
    m0 = blk * (1.0 - np.eye(P, dtype=np.float32))      # denominator mask
    lm0 = ((1.0 - m0) * NEG).astype(np.float32)         # additive log-mask
    pm = np.kron(np.eye(g5, dtype=np.float32), mask24).astype(np.float32)
    im = (TEMP * np.eye(P)).astype(np.float32)   # folds the 1/temperature scale
    ident = np.eye(128, dtype=np.float32).astype(ml_dtypes.bfloat16)
    return {"lm0": lm0, "pm": pm, "im": im, "ident": ident}


def _host_weights():
    """Per-row weights applied on the host: [w1, w2] float64 [P]."""
    i = np.arange(NCLS)
    graph = (np.abs(i[:, None] - i[None, :]) <= 1).astype(np.float64)
    mask24 = np.tile(graph, (NV, NV)) * (1.0 - np.eye(M))
    msum = np.tile(mask24.sum(1), P // M)                # [120], 3 or 5
    alpha = -TEMP / ((msum + EPS_POS) * M * B)           # per-row weight
    return alpha, -alpha * msum


def _build():
    from contextlib import ExitStack

    from concourse import bacc, bass, mybir, tile

    f32 = mybir.dt.float32
    bf16 = mybir.dt.bfloat16
    AX = mybir.AxisListType
    ALU = mybir.AluOpType
    ACTF = mybir.ActivationFunctionType

    import bass_rust as _bass_rust
    from concourse.hw_specs import get_activation_tables

    class _OneActSetBacc(bacc.Bacc):
        """Every activation used here (Ln, Exp, Copy) lives in the
        natural_log_exp_and_others ACT table set; restricting the set-choice
        pass to it means one ACT_TABLE_LOAD for the whole kernel instead of
        ~1.3us switches between the exp- and ln-anchored sets per use.  Other
        sets keep their list position (ids are positional) but advertise no
        functions, so the pass cannot pick them."""

        def insert_act_table_loads(self):
            has_activation = any(
                isinstance(i, mybir.InstActivation)
                for b in self.main_func.blocks
                for i in b.instructions
            )
            if not has_activation:
                return
            tables = [
                (n, (s if n == "natural_log_exp_and_others" else set()))
                for n, s in get_activation_tables(self.m.arch).items()
            ]
            _bass_rust.insert_act_table_loads(self, tables)

    nc = _OneActSetBacc("TRN2", target_bir_lowering=False, debug=False,
                        num_devices=NCORES)

    f_dram = nc.dram_tensor("f", (ROWS, D), f32, kind="ExternalInput")
    lm0_dram = nc.dram_tensor("lm0", (P, P), f32, kind="ExternalInput")
    pm_dram = nc.dram_tensor("pm", (P, P), f32, kind="ExternalInput")
    im_dram = nc.dram_tensor("im", (P, P), f32, kind="ExternalInput")
    id_dram = nc.dram_tensor("ident", (128, 128), bf16, kind="ExternalInput")
    out_dram = nc.dram_tensor("out", (P, 2 * T), f32, kind="ExternalOutput")

    ROWCNT = [P] * (T - 1) + [PTAIL]
    ROWOFF = [P * t for t in range(T)]
    # DMA pieces per tile: one big trigger per full tile, finer pieces for
    # the last two so the end-of-stream completion granularity is small.
    NSPLIT = [1] * (T - 2) + [4, 8]

    with ExitStack() as ctx:
        tc = ctx.enter_context(tile.TileContext(nc))
        consts = ctx.enter_context(tc.tile_pool(name="consts", bufs=1))
        fpool = ctx.enter_context(tc.tile_pool(name="fpool", bufs=T))
        tcpool = ctx.enter_context(tc.tile_pool(name="tcpool", bufs=5))
        work = ctx.enter_context(tc.tile_pool(name="work", bufs=1))
        lwork = ctx.enter_context(tc.tile_pool(name="lwork", bufs=2))
        small = ctx.enter_context(tc.tile_pool(name="small", bufs=2))
        egpool = ctx.enter_context(tc.tile_pool(name="egpool", bufs=4))
        tpsum = ctx.enter_context(
            tc.tile_pool(name="tpsum", bufs=5, space=bass.MemorySpace.PSUM))
        gpsum = ctx.enter_context(
            tc.tile_pool(name="gpsum", bufs=2, space=bass.MemorySpace.PSUM))
        rpsum = ctx.enter_context(
            tc.tile_pool(name="rpsum", bufs=1, space=bass.MemorySpace.PSUM))

        # ALL feature loads lead the program: the gpsimd engine writes the
        # SWDGE descriptors for every tile back-to-back so the DMA engines
        # stream HBM continuously regardless of compute progress.
        ftiles = []
        for t in range(T):
            ft = fpool.tile([P, D], bf16, tag="f")
            r0, rn, nsp = ROWOFF[t], ROWCNT[t], NSPLIT[t]
            w = D // nsp
            for q in range(nsp):
                nc.gpsimd.dma_start(ft[:rn, q * w:(q + 1) * w],
                                    f_dram[r0:r0 + rn, q * w:(q + 1) * w])
            ftiles.append(ft)

        identb = consts.tile([128, 128], bf16, tag="identb")
        lm0_t = consts.tile([P, P], f32, tag="lm0")
        pm_t = consts.tile([P, P], f32, tag="pm")
        im_t = consts.tile([P, P], f32, tag="im")
        nc.scalar.dma_start(identb[:], id_dram[:, :])
        nc.scalar.dma_start(lm0_t[:], lm0_dram[:, :])
        nc.scalar.dma_start(pm_t[:], pm_dram[:, :])
        nc.scalar.dma_start(im_t[:], im_dram[:, :])

        # Preload the exp/ln activation table set while DMA streams.
        warm = consts.tile([1, 2], f32, tag="warm")
        nc.vector.memset(warm[:], 1.0)
        nc.scalar.activation(warm[:, 1:2], warm[:, 0:1], ACTF.Exp)

        # Per-tile results: res[:, t] = sum(pm*logits), res[:, T+t] = ln(denom).
        res = work.tile([P, 2 * T], f32, tag="res")
        nc.vector.memset(res[:], 0.0)

        egs = {}

        def tile_gram(t):
            """Transpose + Gram for tile t; returns d2 = TEMP*diag(G)."""
            ft = ftiles[t]
            rn = ROWCNT[t]
            g = gpsum.tile([P, P], f32, tag="g")
            tcs_list = []
            interleave = (t == T - 1)
            for q in range(NQ):
                tp = tpsum.tile([128, QUAD * P], bf16, tag="tp")
                tcs = tcpool.tile([128, QUAD * P], bf16, tag="tc")
                nhalf = 2 if interleave else 1
                hw = QUAD // nhalf
                for h in range(nhalf):
                    for j in range(h * hw, (h + 1) * hw):
                        c = q * QUAD + j
                        nc.tensor.transpose(
                            tp[:, j * P:j * P + rn],
                            ft[:rn, c * CH:(c + 1) * CH],
                            identb[:rn, :rn],
                        )
                    lo, hi = h * hw * P, ((h + 1) * hw - 1) * P + rn
                    if interleave:
                        # final halves on the (faster) DVE to shorten the tail
                        use_scalar = (2 * q + h) in (0, 2, 4)
                    else:
                        use_scalar = (q == 1) or (q == 3 and t % 2 == 1)
                    if use_scalar:
                        nc.scalar.copy(tcs[:, lo:hi], tp[:, lo:hi])
                    else:
                        nc.vector.tensor_copy(tcs[:, lo:hi], tp[:, lo:hi])
                    if interleave:
                        for j in range(h * hw, (h + 1) * hw):
                            c = q * QUAD + j
                            sl = tcs[:, j * P:j * P + rn]
                            nc.tensor.matmul(g[:rn, :rn], sl, sl,
                                             start=(c == 0), stop=(c == NCH - 1))
                tcs_list.append(tcs)
            if not interleave:
                for c in range(NCH):
                    sl = tcs_list[c // QUAD][:, (c % QUAD) * P:(c % QUAD) * P + rn]
                    nc.tensor.matmul(g[:rn, :rn], sl, sl,
                                     start=(c == 0), stop=(c == NCH - 1))
            eg = egpool.tile([P, P], bf16, tag="eg")
            nc.vector.tensor_copy(eg[:rn, :rn], g[:rn, :rn])
            egs[t] = eg
            # d2 = 0.1 * diag(G)  (im_t is pre-scaled by TEMP)
            scr = lwork.tile([P, P], f32, tag="scr")
            nc.vector.tensor_tensor(scr[:rn, :rn], g[:rn, :rn], im_t[:rn, :rn],
                                    ALU.mult)
            d2 = small.tile([P, 1], f32, tag="d2")
            nc.vector.tensor_reduce(d2[:rn], scr[:rn, :rn], axis=AX.X,
                                    op=ALU.add)
            return d2

        def tile_softmax(t, d2):
            # rnx = (0.1*d2)^-0.5 via exp/ln (same ACT table set);
            # logits L = diag(rnx) @ G @ diag(rnx); the column scaling runs on
            # the PE as G @ diag(rnx) (G is symmetric so lhsT=G is G^T), the
            # row scaling as a per-partition tensor_scalar.
            rn = ROWCNT[t]
            # gpsimd is busy writing SWDGE descriptors until ~tile 4; only
            # hand it SBUF-only ops afterwards (it cannot touch PSUM).
            veng = nc.gpsimd if t >= 5 else nc.vector
            eg = egs.pop(t)
            lnv = small.tile([P, 1], f32, tag="lnv")
            nc.scalar.activation(lnv[:rn], d2[:rn], ACTF.Ln)
            rnx = small.tile([P, 1], f32, tag="rnx")
            nc.scalar.activation(rnx[:rn], lnv[:rn], ACTF.Exp, scale=-0.5)
            drn = lwork.tile([P, P], bf16, tag="drn")
            veng.tensor_scalar(drn[:rn, :rn], im_t[:rn, :rn], rnx[:rn],
                               1.0 / TEMP, op0=ALU.mult, op1=ALU.mult)
            h_ps = rpsum.tile([P, P], f32, tag="r")
            nc.tensor.matmul(h_ps[:rn, :rn], eg[:rn, :rn], drn[:rn, :rn],
                             start=True, stop=True)
            # positive-pair sum: res[:, t] = rnx * sum(pm * h)
            ph = lwork.tile([P, P], f32, tag="ph")
            nc.vector.tensor_tensor(ph[:rn, :rn], h_ps[:rn, :rn],
                                    pm_t[:rn, :rn], ALU.mult)
            t1r = small.tile([P, 1], f32, tag="t1r")
            nc.vector.tensor_reduce(t1r[:rn], ph[:rn, :rn], axis=AX.X,
                                    op=ALU.add)
            veng.tensor_scalar_mul(res[:rn, t:t + 1], t1r[:rn], rnx[:rn])
            # denominator: st = sum(exp(rnx*h + logmask)) fused via accum_out
            lt = lwork.tile([P, P], f32, tag="lt")
            nc.vector.tensor_scalar_mul(lt[:rn, :rn], h_ps[:rn, :rn], rnx[:rn])
            ltm = lwork.tile([P, P], f32, tag="ltm")
            veng.tensor_tensor(ltm[:rn, :rn], lt[:rn, :rn],
                               lm0_t[:rn, :rn], ALU.add)
            xt = lwork.tile([P, P], f32, tag="xt")
            st = small.tile([P, 1], f32, tag="st")
            nc.scalar.activation(xt[:rn, :rn], ltm[:rn, :rn], ACTF.Exp,
                                 accum_out=st[:rn])
            nc.scalar.activation(res[:rn, T + t:T + t + 1], st[:rn], ACTF.Ln)

        for t in range(T):
            d2 = tile_gram(t)
            tile_softmax(t, d2)

        nc.sync.dma_start(out_dram[:, :], res[:])

    nc.compile()
    return nc


def _ensure_axon_hooks():
    """Provide antenv.axon_hooks if the image lacks it (NTFF profiling shim).

    Mirrors trn_agent_boot.trn_boot: the hook drives NRT profiling via the
    libaxon_pjrt.so C ABI.  If anything is missing we register a None hook,
    which makes bass_utils skip tracing gracefully instead of crashing.
    """
    try:
        import antenv.axon_hooks  # noqa: F401
        return
    except ImportError:
        pass
    import contextlib
    import ctypes
    import types

    import antenv

    hook = None
    so_path = "/opt/axon/libaxon_pjrt.so"
    try:
        lib = ctypes.CDLL(so_path)
        if hasattr(lib, "axon_start_nrt_profile"):
            lib.axon_start_nrt_profile.argtypes = [
                ctypes.POINTER(ctypes.c_int64), ctypes.c_size_t]
            lib.axon_start_nrt_profile.restype = ctypes.c_int64
            lib.axon_stop_nrt_profile.argtypes = [ctypes.c_char_p]
            lib.axon_stop_nrt_profile.restype = ctypes.c_int64

            @contextlib.contextmanager
            def _hook(output_dir, device_ids):
                import jax
                jax.devices()
                if device_ids:
                    ids = (ctypes.c_int64 * len(device_ids))(*device_ids)
                    rc = lib.axon_start_nrt_profile(ids, len(device_ids))
                else:
                    rc = lib.axon_start_nrt_profile(None, 0)
                if rc != 0:
                    raise RuntimeError(f"axon_start_nrt_profile rc={rc}")
                try:
                    yield
                finally:
                    n = lib.axon_stop_nrt_profile(str(output_dir).encode())
                    print(f"profile: {n} file(s) written to {output_dir}",
                          file=sys.stderr)

            hook = _hook
    except OSError:
        pass

    mod = types.ModuleType("antenv.axon_hooks")
    state = {"hook": hook}
    mod.get_axon_ntff_profile_hook = lambda: state["hook"]
    mod.set_axon_ntff_profile_hook = lambda h: state.__setitem__("hook", h)
    sys.modules["antenv.axon_hooks"] = mod
    antenv.axon_hooks = mod


def kernel(**inputs):
    global _compiled, LAST_RESULTS
    from concourse import bass_utils

    feats = np.ascontiguousarray(
        np.asarray(inputs["features"], dtype=np.float32).reshape(B * M, D))

    if _compiled is None:
        _compiled = (_build(), _host_consts())
    nc, consts = _compiled

    in_maps = []
    for k in range(NCORES):
        im = dict(consts)
        im["f"] = feats[k * ROWS:(k + 1) * ROWS]
        in_maps.append(im)

    trace = bool(os.environ.get("BASS_TRACE"))
    if trace:
        _ensure_axon_hooks()
    try:
        res = bass_utils.run_bass_kernel_spmd(
            nc, in_maps, core_ids=list(range(NCORES)), trace=trace)
    except Exception:
        # Tracing plumbing or a transient device hiccup; retry once untraced.
        os.environ["BASS_NEVER_TRACE"] = "1"
        try:
            res = bass_utils.run_bass_kernel_spmd(
                nc, in_maps, core_ids=list(range(NCORES)), trace=False)
        finally:
            del os.environ["BASS_NEVER_TRACE"]
    LAST_RESULTS = res
    w1, w2 = _host_weights()
    total = 0.0
    for r in res.results:
        o = np.asarray(r["out"], dtype=np.float64)
        total += w1 @ o[:, :T].sum(axis=1) + w2 @ o[:, T:].sum(axis=1)
    return np.array(total, dtype=np.float32)
